# revision 27
# baseline (speedup 1.0000x reference)
"""Trainium2 Bass kernel for nn_CSA_36971078484033.

Instance-norm over (H,W) per (B,C) with a Dirichlet-weighted prototype affine
(label-conditional bank selection), data-parallel over B on 8 NeuronCores.

  out[b,c,h,w] = (x[b,c,h,w] - mean[b,c]) / sqrt(var[b,c] + eps) * new_std[b,c]
               + new_mean[b,c]
  new_mean = (label==0) ? w@proto_mean_pos : w@proto_mean_neg   (same for std)

Per core: 4 samples = 8 tiles of [128ch, 3136px].  The tiny [64,4]x[64,256]
prototype einsum runs on TensorE with the label selection folded into
host-masked weights (w*(label==0) and w*(label!=0) contribute to pos/neg
banks; the unselected bank's weights are zero).

x and y stream through HBM as bf16 (converted host-side; stats and the affine
math stay f32 on-chip); bf16 round-trip costs ~5e-3 rel err vs the 2e-2 gate.
At bf16 the 12.8 MB/core stream costs ~36us of DMA, so every compute engine
must stay under that.  bn_stats runs at 1x (~4.5us/tile -> 38us: too slow),
so stats come from two accumulate passes instead:
  sum(x):   DVE tensor_scalar copy in 4x bf16 perf mode with accum_out
  sum(x^2): ScalarE Square activation with accum_out
and the affine apply y = x*scl - shf' is a third 4x DVE tensor_scalar.
DVE ~3us/tile and ScalarE ~3us/tile both hide under the ~4.5us/tile DMA.
A 1-tile-deep software pipeline (variance/sqrt/apply of tile i emitted after
the accum passes of tile i+1) keeps the in-order engines from stalling on
cross-engine semaphore waits.
"""

import numpy as np
from contextlib import ExitStack

B, C, H, W = 32, 256, 56, 56
HW = H * W            # 3136
K = 64
EPS = 1e-5
NCORES = 8
BPC = B // NCORES     # 4 samples per core
ROWS = BPC * C        # 1024 DRAM rows per core
PCOLS = 2 * 4 + 4 * 256   # wposT|wnegT|pmp|psp|pmn|psn packed columns

_cache = {}


def _emit(tc, nc, mybir, aps):
    f32 = mybir.dt.float32
    bf16 = mybir.dt.bfloat16
    x_d, packed_d, y_d = aps
    with ExitStack() as ctx:
        consts = ctx.enter_context(tc.tile_pool(name="consts", bufs=1))
        xpool = ctx.enter_context(tc.tile_pool(name="xp", bufs=8))
        ypool = ctx.enter_context(tc.tile_pool(name="yp", bufs=6))
        stats = ctx.enter_context(tc.tile_pool(name="stats", bufs=5))
        psum = ctx.enter_context(tc.tile_pool(name="psum", bufs=2, space="PSUM"))

        # --- tiny inputs packed host-side into ONE [64, 1032] tensor:
        # a single HWDGE push (~0.6us) instead of six (~3.6us); on the Sync
        # ring ahead of the x stream so ScalarE never touches a DMA ---
        packed_sb = consts.tile([K, PCOLS], f32, tag="packed")
        nc.sync.dma_start(packed_sb[:], packed_d[:])
        wpos_sb = packed_sb[:, 0:BPC]
        wneg_sb = packed_sb[:, BPC:2 * BPC]
        protos = {}
        for i, name in enumerate(("pmp", "psp", "pmn", "psn")):
            protos[name] = packed_sb[:, 2 * BPC + i * C: 2 * BPC + (i + 1) * C]

        eps_sb = consts.tile([128, 1], f32, tag="eps")
        nc.gpsimd.memset(eps_sb[:], EPS)

        # --- selected new_mean/new_std, channel-major: [128ch, BPC] per half.
        # Matmuls emit now (TensorE is free); the PSUM->SBUF copies are
        # deferred into the group loop so DVE's in-order stream starts on the
        # first adder tree instead of stalling behind the matmul prologue ---
        mean_sel = consts.tile([128, 2 * BPC], f32, tag="mean_sel")
        std_sel = consts.tile([128, 2 * BPC], f32, tag="std_sel")
        sel_copies = []
        for h in range(2):
            cs = slice(h * 128, (h + 1) * 128)
            bs = slice(h * BPC, (h + 1) * BPC)
            pm = psum.tile([128, BPC], f32, tag="ps_mm")
            nc.tensor.matmul(pm[:], protos["pmp"][:, cs], wpos_sb, start=True, stop=False)
            nc.tensor.matmul(pm[:], protos["pmn"][:, cs], wneg_sb, start=False, stop=True)
            sel_copies.append((mean_sel[:, bs], pm))
            ps = psum.tile([128, BPC], f32, tag="ps_ss")
            nc.tensor.matmul(ps[:], protos["psp"][:, cs], wpos_sb, start=True, stop=False)
            nc.tensor.matmul(ps[:], protos["psn"][:, cs], wneg_sb, start=False, stop=True)
            sel_copies.append((std_sel[:, bs], ps))

        NT = BPC * 2          # 8 tiles
        # asymmetric groups: singleton first groups get their stats chain,
        # apply, and out-DMA going as early as possible so the out-queue
        # starts draining ~12us sooner; tails batched in pairs
        GROUPS = [[0], [1], [2, 3], [4, 5], [6, 7]]
        Alu = mybir.AluOpType
        AF = mybir.ActivationFunctionType
        N = float(HW)
        K1 = N / (N - 1.0)    # msq_s = (mean*K1)*mean
        K2 = 1.0 / (N - 1.0)  # varu  = sumsq*K2 - msq_s

        # engine-local scratches, reused across tiles (same-engine WAW only,
        # so no semaphores): ScalarE square output + DVE adder-tree levels
        # (tree scratches sized for a 2-tile group; singletons use [:, 0, :])
        scrpool = ctx.enter_context(tc.tile_pool(name="scr", bufs=1))
        scrS = scrpool.tile([128, HW], bf16, tag="scrS")
        h1 = scrpool.tile([128, 2, HW // 2], bf16, tag="h1")
        h2 = scrpool.tile([128, 2, HW // 4], bf16, tag="h2")
        h3 = scrpool.tile([128, 2, HW // 8], bf16, tag="h3")

        # preload the Sqrt activation table before the Square backbone starts:
        # a mid-stream ACT_TABLE_LOAD would stall ScalarE for 1.3us
        warm = stats.tile([128, 1], f32, tag="warm")
        nc.scalar.activation(warm[:], eps_sb[:], AF.Sqrt, bias=eps_sb[:])

        # all in-DMAs up front: the Sync ring does nothing else early, so the
        # in-stream runs at full rate from the start.  2-tile groups load into
        # one [128, 2, HW] tile so their adder trees pair into single
        # instructions (halves the DVE instruction overhead).
        xviews = {}
        for g, tiles in enumerate(GROUPS):
            if len(tiles) == 1:
                x_sb = xpool.tile([128, HW], bf16, tag="xt1")
                views = [x_sb[:]]
            else:
                x_sb = xpool.tile([128, 2, HW], bf16, tag="xt2")
                views = [x_sb[:, i, :] for i in range(2)]
            for i, ti in enumerate(tiles):
                b, h = divmod(ti, 2)
                r0 = b * C + h * 128
                nc.sync.dma_start(views[i], x_d[r0:r0 + 128, :])
                xviews[ti] = views[i]
            if len(tiles) == 2:
                xviews[(g, "pair")] = x_sb

        # sel copies on ScalarE in its idle pre-stream window (matmuls retire
        # ~11us, x0's semaphore lands ~14us) so DVE spends zero time on them
        for dst, src in sel_copies:
            nc.scalar.activation(dst, src[:], AF.Copy)

        sums_g, sumsqs_g = {}, {}

        def accum_group(g):
            # sum(x) via a bf16 pairwise adder tree (packed 2-byte
            # tensor_tensor adds run in the DVE 2x perf mode; a direct
            # tensor_reduce or accum_out pass runs 1x = 3.4us), both tiles of
            # a pair group in one instruction per level; sum(x^2) via ScalarE
            # Square-with-accumulate per tile
            tiles = GROUPS[g]
            GT = len(tiles)
            s = stats.tile([128, GT], f32, tag="sum")
            sq = stats.tile([128, GT], f32, tag="sumsq")
            if GT == 1:
                x_sb = xviews[tiles[0]]
                nc.vector.tensor_add(h1[:, 0, :], x_sb[:, :HW // 2], x_sb[:, HW // 2:])
                nc.vector.tensor_add(h2[:, 0, :], h1[:, 0, :HW // 4], h1[:, 0, HW // 4:])
                nc.vector.tensor_add(h3[:, 0, :], h2[:, 0, :HW // 8], h2[:, 0, HW // 8:])
                nc.vector.tensor_reduce(s[:], h3[:, 0, :],
                                        axis=mybir.AxisListType.X, op=Alu.add)
            else:
                xg = xviews[(g, "pair")]
                nc.vector.tensor_add(h1[:], xg[:, :, :HW // 2], xg[:, :, HW // 2:])
                nc.vector.tensor_add(h2[:], h1[:, :, :HW // 4], h1[:, :, HW // 4:])
                nc.vector.tensor_add(h3[:], h2[:, :, :HW // 8], h2[:, :, HW // 8:])
                nc.vector.tensor_reduce(s[:], h3[:],
                                        axis=mybir.AxisListType.X, op=Alu.add)
            for i, ti in enumerate(tiles):
                nc.scalar.activation(scrS[:], xviews[ti], AF.Square,
                                     accum_out=sq[:, i:i + 1])
            sums_g[g] = s
            sumsqs_g[g] = sq

        def finish_group(g, last=False):
            # chain on DVE ([128, GT] batched stt forms, ~0.15us each); sel
            # tables are [128, 2*BPC] with col = (ti%2)*BPC + ti//2
            tiles = GROUPS[g]
            GT = len(tiles)
            if GT == 1:
                c0 = (tiles[0] % 2) * BPC + tiles[0] // 2
                sel = slice(c0, c0 + 1)
            else:
                sel = slice(tiles[0] // 2, None, BPC)
            mean = stats.tile([128, GT], f32, tag="mean")
            nc.vector.tensor_scalar_mul(mean[:], sums_g[g][:], 1.0 / N)
            msq = stats.tile([128, GT], f32, tag="msq")
            nc.vector.scalar_tensor_tensor(msq[:], mean[:], K1, mean[:],
                                           Alu.mult, Alu.mult)
            varu = stats.tile([128, GT], f32, tag="varu")
            nc.vector.scalar_tensor_tensor(varu[:], sumsqs_g[g][:], K2,
                                           msq[:], Alu.mult, Alu.subtract)
            stdv = stats.tile([128, GT], f32, tag="stdv")
            nc.scalar.activation(stdv[:], varu[:], AF.Sqrt, bias=eps_sb[:])
            rstd = stats.tile([128, GT], f32, tag="rstd")
            nc.vector.reciprocal(rstd[:], stdv[:])
            scl = stats.tile([128, GT], f32, tag="scl")
            nc.vector.tensor_mul(scl[:], rstd[:], std_sel[:, sel])
            # shf' = mean*scl - mean_sel, applied below as y = x*scl - shf'
            shf = stats.tile([128, GT], f32, tag="shf")
            if GT == 1:
                nc.vector.scalar_tensor_tensor(shf[:], mean[:], scl[:],
                                               mean_sel[:, sel],
                                               Alu.mult, Alu.subtract)
            else:
                tmp = stats.tile([128, GT], f32, tag="tmp")
                nc.vector.tensor_mul(tmp[:], mean[:], scl[:])
                nc.vector.tensor_sub(shf[:], tmp[:], mean_sel[:, sel])
            for i, ti in enumerate(tiles):
                b, hh = divmod(ti, 2)
                r0 = b * C + hh * 128
                # apply: 4x-perf-mode DVE tensor_scalar
                y_sb = ypool.tile([128, HW], bf16, tag="yt")
                nc.vector.tensor_scalar(y_sb[:], xviews[ti],
                                        scl[:, i:i + 1], shf[:, i:i + 1],
                                        Alu.mult, Alu.subtract)
                if last:
                    # ScalarE is idle after the last square; its HWDGE queue
                    # drains these two in parallel with the Sync queue's tail
                    nc.scalar.dma_start(y_d[r0:r0 + 128, :], y_sb[:])
                else:
                    # outs ride the Sync HWDGE queue FIFO behind the ins:
                    # in-priority is optimal (it unblocks compute earliest),
                    # the Sync engine is idle after its 9 dispatches, and
                    # avoiding SWDGE saves a fixed ~7us drain at kernel end
                    nc.sync.dma_start(y_d[r0:r0 + 128, :], y_sb[:])

        # 1-group-deep software pipeline: group g's chain+apply emits after
        # group g+1's accumulate passes, so the in-order engines don't stall
        # on cross-engine results that aren't ready yet
        NG = len(GROUPS)
        for g in range(NG):
            accum_group(g)
            if g >= 1:
                finish_group(g - 1)
        finish_group(NG - 1, last=True)


def _program():
    if "nc" in _cache:
        return _cache["nc"]
    import concourse.bass as bass  # noqa: F401
    import concourse.tile as tile
    from concourse import bacc, mybir

    f32 = mybir.dt.float32
    bf16 = mybir.dt.bfloat16
    nc = bacc.Bacc("TRN2", target_bir_lowering=False, debug=False,
                   num_devices=NCORES)
    aps = [
        nc.dram_tensor("x", [ROWS, HW], bf16, kind="ExternalInput").ap(),
        nc.dram_tensor("packed", [K, PCOLS], f32, kind="ExternalInput").ap(),
        nc.dram_tensor("y", [ROWS, HW], bf16, kind="ExternalOutput").ap(),
    ]
    with tile.TileContext(nc) as tc:
        _emit(tc, nc, mybir, aps)
    nc.compile()
    _cache["nc"] = nc
    return nc


def _run(inputs, trace=False, trace_cores=None):
    import ml_dtypes
    from concourse import bass_utils

    nc = _program()

    x = np.asarray(inputs["x"], dtype=np.float32)
    label = np.asarray(inputs["label"])
    w = np.asarray(inputs["combine_weights"], dtype=np.float32)
    pmp = np.ascontiguousarray(np.asarray(inputs["proto_mean_pos"], dtype=np.float32))
    psp = np.ascontiguousarray(np.asarray(inputs["proto_std_pos"], dtype=np.float32))
    pmn = np.ascontiguousarray(np.asarray(inputs["proto_mean_neg"], dtype=np.float32))
    psn = np.ascontiguousarray(np.asarray(inputs["proto_std_neg"], dtype=np.float32))

    is_pos = (label == 0).astype(np.float32)[:, None]   # [B,1]
    wpos = w * is_pos                                   # [B,K]
    wneg = w * (1.0 - is_pos)

    x_bf = x.reshape(NCORES, ROWS, HW).astype(ml_dtypes.bfloat16)
    in_maps = []
    for c in range(NCORES):
        bs = slice(c * BPC, (c + 1) * BPC)
        packed = np.concatenate(
            [wpos[bs].T, wneg[bs].T, pmp, psp, pmn, psn], axis=1)
        in_maps.append({
            "x": np.ascontiguousarray(x_bf[c]),
            "packed": np.ascontiguousarray(packed),
        })

    res = bass_utils.run_bass_kernel_spmd(
        nc, in_maps, core_ids=list(range(NCORES)),
        trace=trace, trace_cores=trace_cores,
    )
    out = np.concatenate(
        [np.asarray(res.results[c]["y"], dtype=np.float32).reshape(BPC, C, H, W)
         for c in range(NCORES)],
        axis=0,
    )
    return out, res


def kernel(**inputs):
    out, _ = _run(inputs, trace=False)
    return out



# revision 31
# speedup vs baseline: 1.0810x; 1.0810x over previous
"""Trainium2 Bass kernel for nn_CSA_36971078484033.

Instance-norm over (H,W) per (B,C) with a Dirichlet-weighted prototype affine
(label-conditional bank selection), data-parallel over B on 8 NeuronCores.

  out[b,c,h,w] = (x[b,c,h,w] - mean[b,c]) / sqrt(var[b,c] + eps) * new_std[b,c]
               + new_mean[b,c]
  new_mean = (label==0) ? w@proto_mean_pos : w@proto_mean_neg   (same for std)

Per core: 4 samples = 8 tiles of [128ch, 3136px].  The tiny [64,4]x[64,256]
prototype einsum runs on TensorE with the label selection folded into
host-masked weights (w*(label==0) and w*(label!=0) contribute to pos/neg
banks; the unselected bank's weights are zero).

x and y stream through HBM as bf16 (converted host-side; stats and the affine
math stay f32 on-chip); bf16 round-trip costs ~5e-3 rel err vs the 2e-2 gate.
At bf16 the 12.8 MB/core stream costs ~36us of DMA, so every compute engine
must stay under that.  bn_stats runs at 1x (~4.5us/tile -> 38us: too slow),
so stats come from two accumulate passes instead:
  sum(x):   DVE tensor_scalar copy in 4x bf16 perf mode with accum_out
  sum(x^2): ScalarE Square activation with accum_out
and the affine apply y = x*scl - shf' is a third 4x DVE tensor_scalar.
DVE ~3us/tile and ScalarE ~3us/tile both hide under the ~4.5us/tile DMA.
A 1-tile-deep software pipeline (variance/sqrt/apply of tile i emitted after
the accum passes of tile i+1) keeps the in-order engines from stalling on
cross-engine semaphore waits.
"""

import numpy as np
from contextlib import ExitStack

B, C, H, W = 32, 256, 56, 56
HW = H * W            # 3136
K = 64
EPS = 1e-5
NCORES = 8
BPC = B // NCORES     # 4 samples per core
ROWS = BPC * C        # 1024 DRAM rows per core
PCOLS = 2 * 4 + 4 * 256   # wposT|wnegT|pmp|psp|pmn|psn packed columns

_cache = {}


def _emit(tc, nc, mybir, aps):
    f32 = mybir.dt.float32
    bf16 = mybir.dt.bfloat16
    x_d, packed_d, y_d = aps
    with ExitStack() as ctx:
        consts = ctx.enter_context(tc.tile_pool(name="consts", bufs=1))
        xpool = ctx.enter_context(tc.tile_pool(name="xp", bufs=8))
        ypool = ctx.enter_context(tc.tile_pool(name="yp", bufs=6))
        stats = ctx.enter_context(tc.tile_pool(name="stats", bufs=5))
        psum = ctx.enter_context(tc.tile_pool(name="psum", bufs=2, space="PSUM"))

        # --- tiny inputs packed host-side into ONE [64, 1032] tensor:
        # a single HWDGE push (~0.6us) instead of six (~3.6us); on the Sync
        # ring ahead of the x stream so ScalarE never touches a DMA ---
        packed_sb = consts.tile([K, PCOLS], f32, tag="packed")
        nc.sync.dma_start(packed_sb[:], packed_d[:])
        wpos_sb = packed_sb[:, 0:BPC]
        wneg_sb = packed_sb[:, BPC:2 * BPC]
        protos = {}
        for i, name in enumerate(("pmp", "psp", "pmn", "psn")):
            protos[name] = packed_sb[:, 2 * BPC + i * C: 2 * BPC + (i + 1) * C]

        eps_sb = consts.tile([128, 1], f32, tag="eps")
        nc.gpsimd.memset(eps_sb[:], EPS)

        # --- selected new_mean/new_std, channel-major: [128ch, BPC] per half.
        # Matmuls emit now (TensorE is free); the PSUM->SBUF copies are
        # deferred into the group loop so DVE's in-order stream starts on the
        # first adder tree instead of stalling behind the matmul prologue ---
        mean_sel = consts.tile([128, 2 * BPC], f32, tag="mean_sel")
        std_sel = consts.tile([128, 2 * BPC], f32, tag="std_sel")
        sel_copies = []
        for h in range(2):
            cs = slice(h * 128, (h + 1) * 128)
            bs = slice(h * BPC, (h + 1) * BPC)
            pm = psum.tile([128, BPC], f32, tag="ps_mm")
            nc.tensor.matmul(pm[:], protos["pmp"][:, cs], wpos_sb, start=True, stop=False)
            nc.tensor.matmul(pm[:], protos["pmn"][:, cs], wneg_sb, start=False, stop=True)
            sel_copies.append((mean_sel[:, bs], pm))
            ps = psum.tile([128, BPC], f32, tag="ps_ss")
            nc.tensor.matmul(ps[:], protos["psp"][:, cs], wpos_sb, start=True, stop=False)
            nc.tensor.matmul(ps[:], protos["psn"][:, cs], wneg_sb, start=False, stop=True)
            sel_copies.append((std_sel[:, bs], ps))

        NT = BPC * 2          # 8 tiles
        # asymmetric groups: singleton first groups get their stats chain,
        # apply, and out-DMA going as early as possible so the out-queue
        # starts draining ~12us sooner; tails batched in pairs
        GROUPS = [[0], [1], [2, 3], [4, 5], [6, 7]]
        Alu = mybir.AluOpType
        AF = mybir.ActivationFunctionType
        N = float(HW)
        K1 = N / (N - 1.0)    # msq_s = (mean*K1)*mean
        K2 = 1.0 / (N - 1.0)  # varu  = sumsq*K2 - msq_s

        # engine-local scratches, reused across tiles (same-engine WAW only,
        # so no semaphores): ScalarE square output + DVE adder-tree levels
        # (tree scratches sized for a 2-tile group; singletons use [:, 0, :])
        scrpool = ctx.enter_context(tc.tile_pool(name="scr", bufs=1))
        scrS = scrpool.tile([128, HW], bf16, tag="scrS")
        h1 = scrpool.tile([128, 2, HW // 2], bf16, tag="h1")
        h2 = scrpool.tile([128, 2, HW // 4], bf16, tag="h2")
        h3 = scrpool.tile([128, 2, HW // 8], bf16, tag="h3")

        # pre-sqrt chain runs on GpSimd (keeps it off DVE's saturated queue),
        # which only supports tensor_tensor ALU forms -> constants as tiles
        invN = consts.tile([128, 2], f32, tag="invN")
        nc.gpsimd.memset(invN[:], 1.0 / N)
        K1c = consts.tile([128, 2], f32, tag="K1c")
        nc.gpsimd.memset(K1c[:], K1)
        K2c = consts.tile([128, 2], f32, tag="K2c")
        nc.gpsimd.memset(K2c[:], K2)

        # preload the Sqrt activation table before the Square backbone starts:
        # a mid-stream ACT_TABLE_LOAD would stall ScalarE for 1.3us
        warm = stats.tile([128, 1], f32, tag="warm")
        nc.scalar.activation(warm[:], eps_sb[:], AF.Sqrt, bias=eps_sb[:])

        # all in-DMAs up front: the Sync ring does nothing else early, so the
        # in-stream runs at full rate from the start.  2-tile groups load into
        # one [128, 2, HW] tile so their adder trees pair into single
        # instructions (halves the DVE instruction overhead).
        xviews = {}
        for g, tiles in enumerate(GROUPS):
            if len(tiles) == 1:
                x_sb = xpool.tile([128, HW], bf16, tag="xt1")
                views = [x_sb[:]]
            else:
                x_sb = xpool.tile([128, 2, HW], bf16, tag="xt2")
                views = [x_sb[:, i, :] for i in range(2)]
            for i, ti in enumerate(tiles):
                b, h = divmod(ti, 2)
                r0 = b * C + h * 128
                nc.sync.dma_start(views[i], x_d[r0:r0 + 128, :])
                xviews[ti] = views[i]
            if len(tiles) == 2:
                xviews[(g, "pair")] = x_sb

        # sel copies early on DVE: matmuls retire ~11us, before the first tree
        for dst, src in sel_copies:
            nc.vector.tensor_copy(dst, src[:])

        sums_g, sumsqs_g = {}, {}

        def accum_group(g):
            # sum(x) via a bf16 pairwise adder tree (packed 2-byte
            # tensor_tensor adds run in the DVE 2x perf mode; a direct
            # tensor_reduce or accum_out pass runs 1x = 3.4us), both tiles of
            # a pair group in one instruction per level; sum(x^2) via ScalarE
            # Square-with-accumulate per tile
            tiles = GROUPS[g]
            GT = len(tiles)
            s = stats.tile([128, GT], f32, tag="sum")
            sq = stats.tile([128, GT], f32, tag="sumsq")
            if GT == 1:
                x_sb = xviews[tiles[0]]
                nc.vector.tensor_add(h1[:, 0, :], x_sb[:, :HW // 2], x_sb[:, HW // 2:])
                nc.vector.tensor_add(h2[:, 0, :], h1[:, 0, :HW // 4], h1[:, 0, HW // 4:])
                nc.vector.tensor_add(h3[:, 0, :], h2[:, 0, :HW // 8], h2[:, 0, HW // 8:])
                nc.vector.tensor_reduce(s[:], h3[:, 0, :],
                                        axis=mybir.AxisListType.X, op=Alu.add)
            else:
                xg = xviews[(g, "pair")]
                nc.vector.tensor_add(h1[:], xg[:, :, :HW // 2], xg[:, :, HW // 2:])
                nc.vector.tensor_add(h2[:], h1[:, :, :HW // 4], h1[:, :, HW // 4:])
                nc.vector.tensor_add(h3[:], h2[:, :, :HW // 8], h2[:, :, HW // 8:])
                nc.vector.tensor_reduce(s[:], h3[:],
                                        axis=mybir.AxisListType.X, op=Alu.add)
            for i, ti in enumerate(tiles):
                nc.scalar.activation(scrS[:], xviews[ti], AF.Square,
                                     accum_out=sq[:, i:i + 1])
            sums_g[g] = s
            sumsqs_g[g] = sq

        def finish_group(g, last=False):
            # chain on DVE ([128, GT] batched stt forms, ~0.15us each); sel
            # tables are [128, 2*BPC] with col = (ti%2)*BPC + ti//2
            tiles = GROUPS[g]
            GT = len(tiles)
            if GT == 1:
                c0 = (tiles[0] % 2) * BPC + tiles[0] // 2
                sel = slice(c0, c0 + 1)
            else:
                sel = slice(tiles[0] // 2, None, BPC)
            # pre-sqrt on GpSimd: relieves DVE (the saturated engine) and
            # decouples sqrt's input from DVE's in-order backlog
            mean = stats.tile([128, GT], f32, tag="mean")
            nc.gpsimd.tensor_mul(mean[:], sums_g[g][:], invN[:, :GT])
            msq = stats.tile([128, GT], f32, tag="msq")
            nc.gpsimd.tensor_mul(msq[:], mean[:], mean[:])
            msqs = stats.tile([128, GT], f32, tag="msqs")
            nc.gpsimd.tensor_mul(msqs[:], msq[:], K1c[:, :GT])
            v0 = stats.tile([128, GT], f32, tag="v0")
            nc.gpsimd.tensor_mul(v0[:], sumsqs_g[g][:], K2c[:, :GT])
            varu = stats.tile([128, GT], f32, tag="varu")
            nc.gpsimd.tensor_sub(varu[:], v0[:], msqs[:])
            stdv = stats.tile([128, GT], f32, tag="stdv")
            nc.scalar.activation(stdv[:], varu[:], AF.Sqrt, bias=eps_sb[:])
            # post-sqrt on DVE (reciprocal is DVE-only)
            rstd = stats.tile([128, GT], f32, tag="rstd")
            nc.vector.reciprocal(rstd[:], stdv[:])
            scl = stats.tile([128, GT], f32, tag="scl")
            nc.vector.tensor_mul(scl[:], rstd[:], std_sel[:, sel])
            # shf' = mean*scl - mean_sel, applied below as y = x*scl - shf'
            shf = stats.tile([128, GT], f32, tag="shf")
            if GT == 1:
                nc.vector.scalar_tensor_tensor(shf[:], mean[:], scl[:],
                                               mean_sel[:, sel],
                                               Alu.mult, Alu.subtract)
            else:
                tmp = stats.tile([128, GT], f32, tag="tmp")
                nc.vector.tensor_mul(tmp[:], mean[:], scl[:])
                nc.vector.tensor_sub(shf[:], tmp[:], mean_sel[:, sel])
            for i, ti in enumerate(tiles):
                b, hh = divmod(ti, 2)
                r0 = b * C + hh * 128
                # apply: 4x-perf-mode DVE tensor_scalar
                y_sb = ypool.tile([128, HW], bf16, tag="yt")
                nc.vector.tensor_scalar(y_sb[:], xviews[ti],
                                        scl[:, i:i + 1], shf[:, i:i + 1],
                                        Alu.mult, Alu.subtract)
                if last:
                    # ScalarE is idle after the last square; its HWDGE queue
                    # drains these two in parallel with the Sync queue's tail
                    nc.scalar.dma_start(y_d[r0:r0 + 128, :], y_sb[:])
                else:
                    # outs ride the Sync HWDGE queue FIFO behind the ins:
                    # in-priority is optimal (it unblocks compute earliest),
                    # the Sync engine is idle after its 9 dispatches, and
                    # avoiding SWDGE saves a fixed ~7us drain at kernel end
                    nc.sync.dma_start(y_d[r0:r0 + 128, :], y_sb[:])

        # 1-group-deep software pipeline: group g's chain+apply emits after
        # group g+1's accumulate passes, so the in-order engines don't stall
        # on cross-engine results that aren't ready yet
        NG = len(GROUPS)
        for g in range(NG):
            accum_group(g)
            if g >= 1:
                finish_group(g - 1)
        finish_group(NG - 1, last=True)


def _program():
    if "nc" in _cache:
        return _cache["nc"]
    import concourse.bass as bass  # noqa: F401
    import concourse.tile as tile
    from concourse import bacc, mybir

    f32 = mybir.dt.float32
    bf16 = mybir.dt.bfloat16
    nc = bacc.Bacc("TRN2", target_bir_lowering=False, debug=False,
                   num_devices=NCORES)
    aps = [
        nc.dram_tensor("x", [ROWS, HW], bf16, kind="ExternalInput").ap(),
        nc.dram_tensor("packed", [K, PCOLS], f32, kind="ExternalInput").ap(),
        nc.dram_tensor("y", [ROWS, HW], bf16, kind="ExternalOutput").ap(),
    ]
    with tile.TileContext(nc) as tc:
        _emit(tc, nc, mybir, aps)
    nc.compile()
    _cache["nc"] = nc
    return nc


def _run(inputs, trace=False, trace_cores=None):
    import ml_dtypes
    from concourse import bass_utils

    nc = _program()

    x = np.asarray(inputs["x"], dtype=np.float32)
    label = np.asarray(inputs["label"])
    w = np.asarray(inputs["combine_weights"], dtype=np.float32)
    pmp = np.ascontiguousarray(np.asarray(inputs["proto_mean_pos"], dtype=np.float32))
    psp = np.ascontiguousarray(np.asarray(inputs["proto_std_pos"], dtype=np.float32))
    pmn = np.ascontiguousarray(np.asarray(inputs["proto_mean_neg"], dtype=np.float32))
    psn = np.ascontiguousarray(np.asarray(inputs["proto_std_neg"], dtype=np.float32))

    is_pos = (label == 0).astype(np.float32)[:, None]   # [B,1]
    wpos = w * is_pos                                   # [B,K]
    wneg = w * (1.0 - is_pos)

    x_bf = x.reshape(NCORES, ROWS, HW).astype(ml_dtypes.bfloat16)
    in_maps = []
    for c in range(NCORES):
        bs = slice(c * BPC, (c + 1) * BPC)
        packed = np.concatenate(
            [wpos[bs].T, wneg[bs].T, pmp, psp, pmn, psn], axis=1)
        in_maps.append({
            "x": np.ascontiguousarray(x_bf[c]),
            "packed": np.ascontiguousarray(packed),
        })

    res = bass_utils.run_bass_kernel_spmd(
        nc, in_maps, core_ids=list(range(NCORES)),
        trace=trace, trace_cores=trace_cores,
    )
    out = np.concatenate(
        [np.asarray(res.results[c]["y"], dtype=np.float32).reshape(BPC, C, H, W)
         for c in range(NCORES)],
        axis=0,
    )
    return out, res


def kernel(**inputs):
    out, _ = _run(inputs, trace=False)
    return out

